# revision 1
# baseline (speedup 1.0000x reference)
"""AAL positional embedding lookup on 8 TRN2 NeuronCores.

Per core (data-parallel over B, 2 batches = 8192 points per core):
  1. DVE pointwise: affine transform (baked at build), round-half-even
     via the 1.5*2^23 magic-add trick (bit-exact with jnp.round), bounds
     mask, clamp, linear voxel index, split into 1KB-block index +
     in-block offset (all exact in f32).
  2. Block ids are put into the SWDGE wrapped-16 idx layout via a masked
     PE matmul (selw/maskw, replication included); one gpsimd dma_gather
     per slice fetches each point's 256-float atlas block from HBM
     (point j's block lands at partition j%128, col j//128).
  3. gpsimd indirect_copy: per 16-partition group, gather each point's
     offset from all 16 channels (the point's own channel is the valid
     one); a static mask + segmented reduce keeps it -> region ids.
  4. One-hot: psB[k, t] = broadcast of region_t over 117 partitions via
     a masked matmul (region*ident, all-ones lhsT); one is_equal op
     against a per-partition iota -> bf16 onehot for 8 chunks at once.
  5. Embedding rows = onehot.T @ table (bf16 matmul, N=512/256);
     PSUM->SBUF copies alternate DVE/ACT into bf16; per-chunk output
     DMAs alternate the two HWDGE engines. Host converts bf16 -> f32
     (values are exactly bf16: one-hot sums of bf16 table rows).

Point layout "PF": point j <-> (partition p = j%128, col c = j//128);
output chunk c = contiguous out rows [128c, 128c+128).
"""

import numpy as np

B, N = 16, 4096
D, H, W = 182, 218, 182
EMBED = 768
REGION_MAX = 116
NCORES = 8
PTS = (B // NCORES) * N          # 8192 points per core
COLS = PTS // 128                # 64
NSLICE = 8
SCOLS = COLS // NSLICE           # 8 cols per slice
SPTS = SCOLS * 128               # 1024 points per slice
BLK = 256                        # atlas block: 256 f32 = 1KB
NBLK = (D * H * W + BLK - 1) // BLK
MAGIC = 12582912.0               # 1.5 * 2^23: RNE rounding trick
FLOOR_C = 0.498046875            # 255/512: floor via round-to-nearest

_cache = {}


def _build(m34):
    import concourse.bacc as bacc
    import concourse.mybir as mybir
    import concourse.tile as tile

    dt = mybir.dt
    Alu = mybir.AluOpType

    is_ident = (
        np.array_equal(m34[:, :3], np.eye(3, dtype=np.float32))
        and np.all(m34[:, 3] == 0.0)
    )

    nc = bacc.Bacc("TRN2", target_bir_lowering=False)

    centers = nc.declare_dram_parameter("centers", [PTS, 3], dt.float32, isOutput=False)
    atlas = nc.declare_dram_parameter("atlas", [NBLK, BLK], dt.float32, isOutput=False)
    table = nc.declare_dram_parameter("table", [REGION_MAX + 1, EMBED], dt.float32, isOutput=False)
    mask16_d = nc.declare_dram_parameter("mask16", [128, SCOLS * 16], dt.float32, isOutput=False)
    iotac_d = nc.declare_dram_parameter("iotac", [128, COLS], dt.float32, isOutput=False)
    iotar_d = nc.declare_dram_parameter("iotar", [128, 1], dt.float32, isOutput=False)
    ident_d = nc.declare_dram_parameter("ident", [128, 128], dt.float32, isOutput=False)
    ones_d = nc.declare_dram_parameter("ones", [128, REGION_MAX + 1], dt.float32, isOutput=False)
    selw_d = nc.declare_dram_parameter("selw", [128, 128], dt.float32, isOutput=False)
    maskw_d = nc.declare_dram_parameter("maskw", [128, COLS * 8], dt.float32, isOutput=False)
    iota256_d = nc.declare_dram_parameter("iota256", [128, BLK], dt.float32, isOutput=False)
    out_d = nc.declare_dram_parameter("out", [PTS, EMBED], dt.bfloat16, isOutput=True)

    with tile.TileContext(nc) as tc:
        with (
            tc.tile_pool(name="const", bufs=1) as cpool,
            tc.tile_pool(name="work", bufs=1) as wpool,
            tc.tile_pool(name="blocks", bufs=4) as bpool,
            tc.tile_pool(name="oh", bufs=2) as ohpool,
            tc.tile_pool(name="osb", bufs=4) as opool,
            tc.tile_pool(name="psB", bufs=1, space="PSUM") as psBp,
            tc.tile_pool(name="psO", bufs=2, space="PSUM") as psOp,
        ):
            # ---- coords FIRST (critical path), in PF layout ----
            coord = []
            for k in range(3):
                t = wpool.tile([128, COLS], dt.float32, tag=f"coord{k}")
                src = centers[:, k : k + 1].rearrange("(c p) one -> p (c one)", p=128)
                nc.sync.dma_start(t[:], src)
                coord.append(t)

            # ---- constants (selw/maskw early on sync; bulk on scalar queue) ----
            selw = cpool.tile([128, 128], dt.float32)
            nc.sync.dma_start(selw[:], selw_d[:, :])
            maskw = cpool.tile([128, COLS * 8], dt.float32)
            nc.sync.dma_start(maskw[:], maskw_d[:, :])
            iotac = cpool.tile([128, COLS], dt.float32)
            nc.sync.dma_start(iotac[:], iotac_d[:, :])
            mask16 = cpool.tile([128, SCOLS * 16], dt.float32)
            nc.scalar.dma_start(mask16[:], mask16_d[:, :])
            iotar = cpool.tile([128, 1], dt.float32)
            nc.scalar.dma_start(iotar[:], iotar_d[:, :])
            iota256 = cpool.tile([128, BLK], dt.float32)
            nc.scalar.dma_start(iota256[:], iota256_d[:, :])
            ident_f = cpool.tile([128, 128], dt.float32)
            nc.scalar.dma_start(ident_f[:], ident_d[:, :])
            ones_f = cpool.tile([128, REGION_MAX + 1], dt.float32)
            nc.scalar.dma_start(ones_f[:], ones_d[:, :])
            ones_bf = cpool.tile([128, REGION_MAX + 1], dt.bfloat16)
            nc.vector.tensor_copy(ones_bf[:], ones_f[:])
            table_f = cpool.tile([REGION_MAX + 1, EMBED], dt.float32)
            nc.scalar.dma_start(table_f[:], table[:, :])
            table_bf = cpool.tile([REGION_MAX + 1, EMBED], dt.bfloat16)
            nc.vector.tensor_copy(table_bf[:], table_f[:])

            # ---- pointwise: transform + round (RNE) ----
            r = []
            for k in range(3):
                rk = wpool.tile([128, COLS], dt.float32, tag=f"r{k}")
                if is_ident:
                    nc.vector.tensor_scalar(
                        rk[:], coord[k][:], MAGIC, MAGIC, op0=Alu.add, op1=Alu.subtract
                    )
                else:
                    t0 = wpool.tile([128, COLS], dt.float32, tag="t0")
                    nc.vector.tensor_scalar_mul(t0[:], coord[0][:], float(m34[k, 0]))
                    nc.vector.scalar_tensor_tensor(
                        t0[:], coord[1][:], float(m34[k, 1]), t0[:],
                        op0=Alu.mult, op1=Alu.add,
                    )
                    nc.vector.scalar_tensor_tensor(
                        t0[:], coord[2][:], float(m34[k, 2]), t0[:],
                        op0=Alu.mult, op1=Alu.add,
                    )
                    nc.vector.tensor_scalar_add(t0[:], t0[:], float(m34[k, 3]))
                    nc.vector.tensor_scalar(
                        rk[:], t0[:], MAGIC, MAGIC, op0=Alu.add, op1=Alu.subtract
                    )
                r.append(rk)

            # ---- clamp + linear index (exact in f32) ----
            lim = [D - 1, H - 1, W - 1]
            c3 = []
            for k in range(3):
                ck = wpool.tile([128, COLS], dt.float32, tag=f"c{k}")
                nc.vector.tensor_scalar(
                    ck[:], r[k][:], 0.0, float(lim[k]), op0=Alu.max, op1=Alu.min
                )
                c3.append(ck)
            lin = wpool.tile([128, COLS], dt.float32, tag="lin")
            nc.vector.scalar_tensor_tensor(
                lin[:], c3[1][:], float(W), c3[2][:], op0=Alu.mult, op1=Alu.add
            )
            nc.vector.scalar_tensor_tensor(
                lin[:], c3[0][:], float(H * W), lin[:], op0=Alu.mult, op1=Alu.add
            )

            # ---- block id (floor(lin/256)) + in-block offset ----
            blockf = wpool.tile([128, COLS], dt.float32, tag="blockf")
            nc.vector.tensor_scalar(
                blockf[:], lin[:], 1.0 / BLK, FLOOR_C, op0=Alu.mult, op1=Alu.subtract
            )
            nc.vector.tensor_scalar(
                blockf[:], blockf[:], MAGIC, MAGIC, op0=Alu.add, op1=Alu.subtract
            )
            # ---- block ids -> wrapped-16 int16, replicated x8, via PE ----
            # wrap16[16g+q, 8c+u] = blockf[16u+q, c]:
            #   rhsW[p, 8c+u] = blockf[p, c] * (p//16 == u)   (maskw)
            #   out = selw.T @ rhsW with selw[p, m] = (p%16 == m%16)
            rhsW = wpool.tile([128, COLS, 8], dt.float32, tag="rhsW")
            nc.vector.tensor_tensor(
                rhsW[:],
                blockf[:]
                .rearrange("p (c one) -> p c one", one=1)
                .to_broadcast([128, COLS, 8]),
                maskw[:].rearrange("p (c u) -> p c u", u=8),
                op=Alu.mult,
            )
            psW = psBp.tile([128, COLS * 8], dt.float32, tag="psB")
            nc.tensor.matmul(
                psW[:], selw[:], rhsW[:].rearrange("p c u -> p (c u)")
            )
            blk_idx = wpool.tile([128, COLS * 8], dt.int16, tag="blk_idx")
            nc.vector.tensor_copy(blk_idx[:], psW[:])

            # ---- bounds mask ----
            valid = wpool.tile([128, COLS], dt.float32, tag="valid")
            nc.vector.tensor_scalar(valid[:], r[0][:], 0.0, None, op0=Alu.is_ge)
            for k in range(3):
                if k > 0:
                    nc.vector.scalar_tensor_tensor(
                        valid[:], r[k][:], 0.0, valid[:], op0=Alu.is_ge, op1=Alu.mult
                    )
                nc.vector.scalar_tensor_tensor(
                    valid[:], r[k][:], float(lim[k]), valid[:],
                    op0=Alu.is_le, op1=Alu.mult,
                )

            off = wpool.tile([128, COLS], dt.float32, tag="off")
            nc.vector.scalar_tensor_tensor(
                off[:], blockf[:], float(-BLK), lin[:], op0=Alu.mult, op1=Alu.add
            )
            # extraction index for indirect_copy: e = off + 256*(c % SCOLS)
            ef = wpool.tile([128, COLS], dt.float32, tag="ef")
            nc.vector.tensor_tensor(ef[:], off[:], iotac[:], op=Alu.add)
            e16 = wpool.tile([128, COLS], dt.uint16, tag="e16")
            nc.vector.tensor_copy(e16[:], ef[:])

            # ---- per-slice pipeline ----
            def emit_gather(s):
                blocks = bpool.tile([128, SCOLS, BLK], dt.float32, tag="blocks")
                nc.gpsimd.dma_gather(
                    blocks[:],
                    atlas[:, :],
                    blk_idx[:, 64 * s : 64 * (s + 1)],
                    SPTS,
                    SPTS,
                    BLK,
                )
                return blocks

            def emit_slice(s, blocks):
                csl = slice(SCOLS * s, SCOLS * (s + 1))
                region = bpool.tile([128, SCOLS], dt.float32, tag="region")
                if s < 8:
                    # gpsimd: 16-candidate gather + masked reduce
                    cand = bpool.tile([128, SCOLS * 16], dt.float32, tag="cand")
                    nc.gpsimd.indirect_copy(
                        cand[:],
                        blocks[:].rearrange("p a b -> p (a b)"),
                        e16[:, csl],
                        True,
                    )
                    candm = bpool.tile([128, SCOLS * 16], dt.float32, tag="candm")
                    nc.vector.tensor_tensor(
                        candm[:], cand[:], mask16[:, :], op=Alu.mult
                    )
                    nc.vector.tensor_reduce(
                        region[:],
                        candm[:].rearrange("p (a b) -> p a b", b=16),
                        axis=mybir.AxisListType.X,
                        op=Alu.add,
                    )
                else:
                    # ACT/DVE: broadcast compare + mult + segmented reduce
                    emask = bpool.tile([128, SCOLS, BLK], dt.float32, tag="emask")
                    nc.any.tensor_tensor(
                        emask[:],
                        iota256[:]
                        .rearrange("p (one e) -> p one e", one=1)
                        .to_broadcast([128, SCOLS, BLK]),
                        off[:, csl]
                        .rearrange("p (c one) -> p c one", one=1)
                        .to_broadcast([128, SCOLS, BLK]),
                        op=Alu.is_equal,
                    )
                    eprod = bpool.tile([128, SCOLS, BLK], dt.float32, tag="eprod")
                    nc.vector.tensor_tensor(
                        eprod[:], blocks[:], emask[:], op=Alu.mult
                    )
                    nc.vector.tensor_reduce(
                        region[:],
                        eprod[:],
                        axis=mybir.AxisListType.X,
                        op=Alu.add,
                    )
                nc.vector.tensor_tensor(
                    region[:], region[:], valid[:, csl], op=Alu.mult
                )

                # broadcast region over 117 partitions via masked matmul:
                # rhsB[p', (cc,p)] = region[p', cc] * (p' == p); psB = 1.T @ rhsB
                rhsB = bpool.tile([128, SCOLS, 128], dt.bfloat16, tag="rhsB")
                nc.vector.tensor_tensor(
                    rhsB[:],
                    region[:]
                    .rearrange("p (c one) -> p c one", one=1)
                    .to_broadcast([128, SCOLS, 128]),
                    ident_f[:]
                    .rearrange("p (one q) -> p one q", one=1)
                    .to_broadcast([128, SCOLS, 128]),
                    op=Alu.mult,
                )
                psB = psBp.tile([REGION_MAX + 1, SPTS], dt.float32, tag="psB")
                rb = rhsB[:].rearrange("p c q -> p (c q)")
                nc.tensor.matmul(psB[:, 0:512], ones_bf[:, :], rb[:, 0:512])
                nc.tensor.matmul(psB[:, 512:1024], ones_bf[:, :], rb[:, 512:1024])
                oh = ohpool.tile([REGION_MAX + 1, SPTS], dt.bfloat16, tag="oh")
                nc.any.tensor_scalar(
                    oh[:], psB[:], iotar[0 : REGION_MAX + 1, :], None,
                    op0=Alu.is_equal,
                )

                for cc in range(SCOLS):
                    c = SCOLS * s + cc
                    lhs = oh[:, 128 * cc : 128 * (cc + 1)]
                    psO = psOp.tile([128, EMBED], dt.float32, tag="psO")
                    nc.tensor.matmul(psO[:, 0:512], lhs, table_bf[:, 0:512])
                    nc.tensor.matmul(psO[:, 512:768], lhs, table_bf[:, 512:768])
                    osb = opool.tile([128, EMBED], dt.bfloat16, tag="osb")
                    if c % 2 == 0:
                        nc.vector.tensor_copy(osb[:], psO[:])
                    else:
                        nc.scalar.copy(osb[:], psO[:])
                    if c >= 40:
                        eng = nc.gpsimd
                    else:
                        eng = nc.sync if c % 2 == 0 else nc.scalar
                    eng.dma_start(out_d[128 * c : 128 * (c + 1), :], osb[:])

            LEAD = 2
            blocks_q = {}
            for s in range(NSLICE + LEAD):
                if s < NSLICE:
                    blocks_q[s] = emit_gather(s)
                if s >= LEAD:
                    emit_slice(s - LEAD, blocks_q.pop(s - LEAD))

    nc.compile()
    return nc


def _consts():
    mask16 = np.zeros((128, 16), dtype=np.float32)
    for p in range(128):
        mask16[p, p % 16] = 1.0
    mask16 = np.tile(mask16, (1, SCOLS))
    iotac = np.tile(
        (np.arange(COLS, dtype=np.float32) % SCOLS) * BLK, (128, 1)
    ).astype(np.float32)
    iotar = np.arange(128, dtype=np.float32).reshape(128, 1)
    ident = np.eye(128, dtype=np.float32)
    ones = np.ones((128, REGION_MAX + 1), dtype=np.float32)
    selw = np.zeros((128, 128), dtype=np.float32)
    for p in range(128):
        for m in range(128):
            if p % 16 == m % 16:
                selw[p, m] = 1.0
    maskw = np.zeros((128, COLS * 8), dtype=np.float32)
    for p in range(128):
        for c in range(COLS):
            maskw[p, 8 * c + (p // 16)] = 1.0
    iota256 = np.tile(np.arange(BLK, dtype=np.float32), (128, 1)).astype(np.float32)
    return mask16, iotac, iotar, ident, ones, selw, maskw, iota256


LAST_RESULTS = None


def kernel(patch_centers_voxels, mri_affine, aal_affine, aal_data, embed_table):
    global LAST_RESULTS
    from concourse.bass_utils import run_bass_kernel_spmd

    pc = np.asarray(patch_centers_voxels, dtype=np.float32)
    mri = np.asarray(mri_affine, dtype=np.float32)
    aal = np.asarray(aal_affine, dtype=np.float32)
    vol = np.asarray(aal_data, dtype=np.float32)
    tab = np.asarray(embed_table, dtype=np.float32)

    minv = np.linalg.inv(aal.astype(np.float32))
    M = (minv @ mri).astype(np.float32)
    m34 = M[:3, :]

    key = m34.tobytes()
    if key not in _cache:
        _cache[key] = _build(m34)
    nc = _cache[key]

    flat = vol.reshape(-1)
    atlas = np.zeros((NBLK * BLK,), dtype=np.float32)
    atlas[: flat.size] = flat
    atlas = atlas.reshape(NBLK, BLK)

    mask16, iotac, iotar, ident, ones, selw, maskw, iota256 = _consts()

    shards = pc.reshape(NCORES, PTS, 3)
    in_maps = []
    for i in range(NCORES):
        in_maps.append(
            {
                "centers": np.ascontiguousarray(shards[i]),
                "atlas": atlas,
                "table": tab,
                "mask16": mask16,
                "iotac": iotac,
                "iotar": iotar,
                "ident": ident,
                "ones": ones,
                "selw": selw,
                "maskw": maskw,
                "iota256": iota256,
            }
        )

    res = run_bass_kernel_spmd(nc, in_maps, core_ids=list(range(NCORES)))
    LAST_RESULTS = res
    out = np.concatenate(
        [np.asarray(res.results[i]["out"]).astype(np.float32) for i in range(NCORES)],
        axis=0,
    )
    return out.reshape(B, N, EMBED)



# revision 5
# speedup vs baseline: 1.2614x; 1.2614x over previous
"""AAL positional embedding lookup on 8 TRN2 NeuronCores.

Per core (data-parallel over B, 2 batches = 8192 points per core):
  1. Centers loaded as ONE contiguous [128, 192] DMA (point j lives at
     partition j//64, col j%64 -- "PC" layout); the affine transform +
     round-half-even (1.5*2^23 magic-add, bit-exact with jnp.round) runs
     on stride-3 views of that tile.  Bounds mask, clamp, linear voxel
     index, 256-voxel block id + in-block offset (all exact in f32).
  2. Block ids -> SWDGE wrapped-16 int16 layout via a masked PE matmul
     (selw/maskw); 8x gpsimd dma_gather, each fetching 1024 points'
     512B bf16 atlas blocks (atlas pre-cast to bf16 on host; region ids
     <= 116 are exact in bf16).  Gather list position i = 128*c + p so
     each point's block lands at its own (partition, col).
  3. Extraction WITHOUT gpsimd: emask = is_equal(iota256, off) (ACT),
     eprod = blocks*emask (DVE), region = reduce_add (DVE).  gpsimd does
     nothing but the 8 gathers (its descriptor generation is the
     critical path at ~9.5ns/point).
  4. One-hot: psB[k, t] = broadcast of region_t over 117 partitions via
     a masked matmul (region*ident, all-ones lhsT); is_equal against a
     per-partition iota -> bf16 onehot for 8 chunks at once.
  5. Embedding rows = onehot.T @ table (bf16 matmul, N=512/256);
     PSUM->SBUF copies alternate DVE/ACT into bf16; per-chunk output
     DMAs go out on the sync/scalar HWDGE queues (never gpsimd).
     Output chunk c = out rows {64p + c} (row stride 64), 1536B/row.
     Host converts bf16 -> f32 (values are exactly bf16).
"""

import numpy as np

B, N = 16, 4096
D, H, W = 182, 218, 182
EMBED = 768
REGION_MAX = 116
NCORES = 8
PTS = (B // NCORES) * N          # 8192 points per core
COLS = PTS // 128                # 64
NSLICE = 8
SCOLS = COLS // NSLICE           # 8 cols per slice
SPTS = SCOLS * 128               # 1024 points per slice
BLK = 256                        # atlas block: 256 bf16 = 512B
NBLK = (D * H * W + BLK - 1) // BLK
MAGIC = 12582912.0               # 1.5 * 2^23: RNE rounding trick
FLOOR_C = 0.498046875            # 255/512: floor via round-to-nearest

_cache = {}


def _build(m34):
    import concourse.bacc as bacc
    import concourse.mybir as mybir
    import concourse.tile as tile

    dt = mybir.dt
    Alu = mybir.AluOpType

    is_ident = (
        np.array_equal(m34[:, :3], np.eye(3, dtype=np.float32))
        and np.all(m34[:, 3] == 0.0)
    )

    nc = bacc.Bacc("TRN2", target_bir_lowering=False)

    centers = nc.declare_dram_parameter("centers", [PTS, 3], dt.float32, isOutput=False)
    atlas = nc.declare_dram_parameter("atlas", [NBLK, BLK], dt.bfloat16, isOutput=False)
    table = nc.declare_dram_parameter("table", [REGION_MAX + 1, EMBED], dt.bfloat16, isOutput=False)
    iotar_d = nc.declare_dram_parameter("iotar", [128, 1], dt.float32, isOutput=False)
    ident_d = nc.declare_dram_parameter("ident", [128, 128], dt.float32, isOutput=False)
    ones_d = nc.declare_dram_parameter("ones", [128, REGION_MAX + 1], dt.bfloat16, isOutput=False)
    selw_d = nc.declare_dram_parameter("selw", [128, 128], dt.float32, isOutput=False)
    maskw_d = nc.declare_dram_parameter("maskw", [128, COLS * 8], dt.float32, isOutput=False)
    iota256_d = nc.declare_dram_parameter("iota256", [128, BLK], dt.bfloat16, isOutput=False)
    out_d = nc.declare_dram_parameter("out", [PTS, EMBED], dt.bfloat16, isOutput=True)

    # out rows {64p + c} for chunk c: [128, 64*768] view, col-slice per chunk
    out_v = out_d[:, :].rearrange("(p c) e -> p (c e)", p=128)

    with tile.TileContext(nc) as tc:
        with (
            tc.tile_pool(name="const", bufs=1) as cpool,
            tc.tile_pool(name="work", bufs=1) as wpool,
            tc.tile_pool(name="blocks", bufs=8) as bpool,
            tc.tile_pool(name="sl", bufs=2) as spool,
            tc.tile_pool(name="oh", bufs=2) as ohpool,
            tc.tile_pool(name="osb", bufs=4) as opool,
            tc.tile_pool(name="psB", bufs=1, space="PSUM") as psBp,
            tc.tile_pool(name="psO", bufs=2, space="PSUM") as psOp,
        ):
            # ---- centers FIRST (critical path): one contiguous DMA ----
            cent = wpool.tile([128, COLS * 3], dt.float32, tag="cent")
            nc.sync.dma_start(
                cent[:], centers[:, :].rearrange("(p c) k -> p (c k)", p=128)
            )
            # strided views: coord k = cent[:, k::3]
            cent3 = cent[:].rearrange("p (c k) -> p k c", k=3)
            coord = [
                cent3[:, k : k + 1, :].rearrange("p one c -> p (one c)")
                for k in range(3)
            ]

            # ---- constants (selw/maskw early on sync; bulk on scalar) ----
            selw = cpool.tile([128, 128], dt.float32)
            nc.sync.dma_start(selw[:], selw_d[:, :])
            maskw = cpool.tile([128, COLS * 8], dt.float32)
            nc.sync.dma_start(maskw[:], maskw_d[:, :])
            iotar = cpool.tile([128, 1], dt.float32)
            nc.scalar.dma_start(iotar[:], iotar_d[:, :])
            iota256 = cpool.tile([128, BLK], dt.bfloat16)
            nc.scalar.dma_start(iota256[:], iota256_d[:, :])
            ident_f = cpool.tile([128, 128], dt.float32)
            nc.scalar.dma_start(ident_f[:], ident_d[:, :])
            ones_bf = cpool.tile([128, REGION_MAX + 1], dt.bfloat16)
            nc.scalar.dma_start(ones_bf[:], ones_d[:, :])
            table_bf = cpool.tile([REGION_MAX + 1, EMBED], dt.bfloat16)
            nc.scalar.dma_start(table_bf[:], table[:, :])

            # ---- pointwise: transform + round (RNE) ----
            r = []
            for k in range(3):
                rk = wpool.tile([128, COLS], dt.float32, tag=f"r{k}")
                if is_ident:
                    nc.vector.tensor_scalar(
                        rk[:], coord[k], MAGIC, MAGIC, op0=Alu.add, op1=Alu.subtract
                    )
                else:
                    t0 = wpool.tile([128, COLS], dt.float32, tag="t0")
                    nc.vector.tensor_scalar_mul(t0[:], coord[0], float(m34[k, 0]))
                    nc.vector.scalar_tensor_tensor(
                        t0[:], coord[1], float(m34[k, 1]), t0[:],
                        op0=Alu.mult, op1=Alu.add,
                    )
                    nc.vector.scalar_tensor_tensor(
                        t0[:], coord[2], float(m34[k, 2]), t0[:],
                        op0=Alu.mult, op1=Alu.add,
                    )
                    nc.vector.tensor_scalar_add(t0[:], t0[:], float(m34[k, 3]))
                    nc.vector.tensor_scalar(
                        rk[:], t0[:], MAGIC, MAGIC, op0=Alu.add, op1=Alu.subtract
                    )
                r.append(rk)

            # ---- clamp + linear index (exact in f32) ----
            lim = [D - 1, H - 1, W - 1]
            c3 = []
            for k in range(3):
                ck = wpool.tile([128, COLS], dt.float32, tag=f"c{k}")
                nc.vector.tensor_scalar(
                    ck[:], r[k][:], 0.0, float(lim[k]), op0=Alu.max, op1=Alu.min
                )
                c3.append(ck)
            lin = wpool.tile([128, COLS], dt.float32, tag="lin")
            nc.vector.scalar_tensor_tensor(
                lin[:], c3[1][:], float(W), c3[2][:], op0=Alu.mult, op1=Alu.add
            )
            nc.vector.scalar_tensor_tensor(
                lin[:], c3[0][:], float(H * W), lin[:], op0=Alu.mult, op1=Alu.add
            )

            # ---- block id (floor(lin/256)) + in-block offset ----
            blockf = wpool.tile([128, COLS], dt.float32, tag="blockf")
            nc.vector.tensor_scalar(
                blockf[:], lin[:], 1.0 / BLK, FLOOR_C, op0=Alu.mult, op1=Alu.subtract
            )
            nc.vector.tensor_scalar(
                blockf[:], blockf[:], MAGIC, MAGIC, op0=Alu.add, op1=Alu.subtract
            )
            # ---- block ids -> wrapped-16 int16, replicated x8, via PE ----
            # wrap16[16g+q, 8c+u] = blockf[16u+q, c]:
            #   rhsW[p, 8c+u] = blockf[p, c] * (p//16 == u)   (maskw)
            #   out = selw.T @ rhsW with selw[p, m] = (p%16 == m%16)
            rhsW = wpool.tile([128, COLS, 8], dt.float32, tag="rhsW")
            nc.vector.tensor_tensor(
                rhsW[:],
                blockf[:]
                .rearrange("p (c one) -> p c one", one=1)
                .to_broadcast([128, COLS, 8]),
                maskw[:].rearrange("p (c u) -> p c u", u=8),
                op=Alu.mult,
            )
            psW = psBp.tile([128, COLS * 8], dt.float32, tag="psW")
            nc.tensor.matmul(
                psW[:], selw[:], rhsW[:].rearrange("p c u -> p (c u)")
            )
            blk_idx = wpool.tile([128, COLS * 8], dt.int16, tag="blk_idx")
            nc.vector.tensor_copy(blk_idx[:], psW[:])

            # ---- bounds mask ----
            valid = wpool.tile([128, COLS], dt.float32, tag="valid")
            nc.vector.tensor_scalar(valid[:], r[0][:], 0.0, None, op0=Alu.is_ge)
            for k in range(3):
                if k > 0:
                    nc.vector.scalar_tensor_tensor(
                        valid[:], r[k][:], 0.0, valid[:], op0=Alu.is_ge, op1=Alu.mult
                    )
                nc.vector.scalar_tensor_tensor(
                    valid[:], r[k][:], float(lim[k]), valid[:],
                    op0=Alu.is_le, op1=Alu.mult,
                )

            # in-block offset in bf16 (0..255 exact) for the extraction mask
            off = wpool.tile([128, COLS], dt.float32, tag="off")
            nc.vector.scalar_tensor_tensor(
                off[:], blockf[:], float(-BLK), lin[:], op0=Alu.mult, op1=Alu.add
            )
            off_bf = wpool.tile([128, COLS], dt.bfloat16, tag="off_bf")
            nc.vector.tensor_copy(off_bf[:], off[:])

            # ---- gathers: the only gpsimd work, back-to-back ----
            blocks_q = {}
            for s in range(NSLICE):
                blocks = bpool.tile([128, SCOLS, BLK], dt.bfloat16, tag="blocks")
                nc.gpsimd.dma_gather(
                    blocks[:],
                    atlas[:, :],
                    blk_idx[:, 64 * s : 64 * (s + 1)],
                    SPTS,
                    SPTS,
                    BLK,
                )
                blocks_q[s] = blocks

            # ---- per-slice: extract -> one-hot -> embed matmul -> out ----
            for s in range(NSLICE):
                blocks = blocks_q.pop(s)
                csl = slice(SCOLS * s, SCOLS * (s + 1))
                # extraction on ACT/DVE (no gpsimd)
                emask = spool.tile([128, SCOLS, BLK], dt.bfloat16, tag="emask")
                nc.any.tensor_tensor(
                    emask[:],
                    iota256[:]
                    .rearrange("p (one e) -> p one e", one=1)
                    .to_broadcast([128, SCOLS, BLK]),
                    off_bf[:, csl]
                    .rearrange("p (c one) -> p c one", one=1)
                    .to_broadcast([128, SCOLS, BLK]),
                    op=Alu.is_equal,
                )
                eprod = spool.tile([128, SCOLS, BLK], dt.bfloat16, tag="eprod")
                nc.vector.tensor_tensor(
                    eprod[:], blocks[:], emask[:], op=Alu.mult
                )
                region = spool.tile([128, SCOLS], dt.float32, tag="region")
                nc.vector.tensor_reduce(
                    region[:],
                    eprod[:],
                    axis=mybir.AxisListType.X,
                    op=Alu.add,
                )
                nc.vector.tensor_tensor(
                    region[:], region[:], valid[:, csl], op=Alu.mult
                )

                # broadcast region over 117 partitions via masked matmul:
                # rhsB[p', (cc,p)] = region[p', cc] * (p' == p); psB = 1.T @ rhsB
                rhsB = spool.tile([128, SCOLS, 128], dt.bfloat16, tag="rhsB")
                nc.vector.tensor_tensor(
                    rhsB[:],
                    region[:]
                    .rearrange("p (c one) -> p c one", one=1)
                    .to_broadcast([128, SCOLS, 128]),
                    ident_f[:]
                    .rearrange("p (one q) -> p one q", one=1)
                    .to_broadcast([128, SCOLS, 128]),
                    op=Alu.mult,
                )
                psB = psBp.tile([REGION_MAX + 1, SPTS], dt.float32, tag="psB")
                rb = rhsB[:].rearrange("p c q -> p (c q)")
                nc.tensor.matmul(psB[:, 0:512], ones_bf[:, :], rb[:, 0:512])
                nc.tensor.matmul(psB[:, 512:1024], ones_bf[:, :], rb[:, 512:1024])
                oh = ohpool.tile([REGION_MAX + 1, SPTS], dt.bfloat16, tag="oh")
                nc.any.tensor_scalar(
                    oh[:], psB[:], iotar[0 : REGION_MAX + 1, :], None,
                    op0=Alu.is_equal,
                )

                for cc in range(SCOLS):
                    c = SCOLS * s + cc
                    lhs = oh[:, 128 * cc : 128 * (cc + 1)]
                    psO = psOp.tile([128, EMBED], dt.float32, tag="psO")
                    nc.tensor.matmul(psO[:, 0:512], lhs, table_bf[:, 0:512])
                    nc.tensor.matmul(psO[:, 512:768], lhs, table_bf[:, 512:768])
                    osb = opool.tile([128, EMBED], dt.bfloat16, tag="osb")
                    if c % 2 == 0:
                        nc.vector.tensor_copy(osb[:], psO[:])
                    else:
                        nc.scalar.copy(osb[:], psO[:])
                    eng = nc.scalar if cc >= 6 else nc.sync
                    eng.dma_start(
                        out_v[:, EMBED * c : EMBED * (c + 1)], osb[:]
                    )

    nc.compile()
    return nc


def _consts():
    iotar = np.arange(128, dtype=np.float32).reshape(128, 1)
    ident = np.eye(128, dtype=np.float32)
    ones = np.ones((128, REGION_MAX + 1), dtype=np.float32)
    selw = np.zeros((128, 128), dtype=np.float32)
    for p in range(128):
        for m in range(128):
            if p % 16 == m % 16:
                selw[p, m] = 1.0
    maskw = np.zeros((128, COLS * 8), dtype=np.float32)
    for p in range(128):
        for c in range(COLS):
            maskw[p, 8 * c + (p // 16)] = 1.0
    iota256 = np.tile(np.arange(BLK, dtype=np.float32), (128, 1)).astype(np.float32)
    return iotar, ident, ones, selw, maskw, iota256


LAST_RESULTS = None


def kernel(patch_centers_voxels, mri_affine, aal_affine, aal_data, embed_table):
    global LAST_RESULTS
    import ml_dtypes
    from concourse.bass_utils import run_bass_kernel_spmd

    bf16 = ml_dtypes.bfloat16

    pc = np.asarray(patch_centers_voxels, dtype=np.float32)
    mri = np.asarray(mri_affine, dtype=np.float32)
    aal = np.asarray(aal_affine, dtype=np.float32)
    vol = np.asarray(aal_data, dtype=np.float32)
    tab = np.asarray(embed_table, dtype=np.float32)

    minv = np.linalg.inv(aal.astype(np.float32))
    M = (minv @ mri).astype(np.float32)
    m34 = M[:3, :]

    key = m34.tobytes()
    if key not in _cache:
        _cache[key] = _build(m34)
    nc = _cache[key]

    flat = vol.reshape(-1)
    atlas = np.zeros((NBLK * BLK,), dtype=np.float32)
    atlas[: flat.size] = flat
    atlas = atlas.reshape(NBLK, BLK).astype(bf16)

    tab_bf = tab.astype(bf16)

    iotar, ident, ones, selw, maskw, iota256 = _consts()
    ones_bf = ones.astype(bf16)
    iota256_bf = iota256.astype(bf16)

    shards = pc.reshape(NCORES, PTS, 3)
    in_maps = []
    for i in range(NCORES):
        in_maps.append(
            {
                "centers": np.ascontiguousarray(shards[i]),
                "atlas": atlas,
                "table": tab_bf,
                "iotar": iotar,
                "ident": ident,
                "ones": ones_bf,
                "selw": selw,
                "maskw": maskw,
                "iota256": iota256_bf,
            }
        )

    res = run_bass_kernel_spmd(nc, in_maps, core_ids=list(range(NCORES)))
    LAST_RESULTS = res
    outs = []
    for i in range(NCORES):
        o = np.asarray(res.results[i]["out"]).astype(np.float32)
        # undo PC layout: row of chunk c at out[64p + c] is point p*64+c...
        # out_d rows are already point order: chunk c wrote rows {64p+c}
        # which IS row index j = 64p + c = point (p, c) -> identity.
        outs.append(o)
    out = np.concatenate(outs, axis=0)
    return out.reshape(B, N, EMBED)
